# revision 31
# baseline (speedup 1.0000x reference)
"""AdaptiveMLP Trainium2 kernel (8-core data parallel), v4.

Math per layer: y[b,o] = sum_{n,i} co[b,n]*x[b,i]*W[n,i,o] + sum_n co[b,n]*b[n,o]

Feature-major chain per core (B_LOC=8192 samples; column c <-> sample
b = (c%128)*64 + c//128):
  - u0coT [42, B]: rows n*3+i = co_n*x_i, rows 32+n = co_n (DVE mult of
    host-prepared xrep/corep).
  - L0: one matmul per 512-chunk with stationary [W0flat | W0flat]
    [42, 128] -> psum [128, 512] holds z1 TWICE; Act relu -> x1dup.
    Pair-4 cb broadcast (sel2 matmul from compact coT) woven between
    L0 chunks; evictions split Act/DVE.
  - L1 per 2048-block: xp_p = x1dup * cb_p (DVE); per [128,512] psum
    bank (2 chunks via tile_position): B1 bias matmul + 5 pair matmuls;
    pair 3 (the last DMA arrival) accumulated LAST so the tail after
    the final cb byte is only 4 matmuls + the L2 chain. relu -> x2dual.
  - L2 per block: pt2 = 4x W2 matmuls; Act Identity eviction adds the
    constant b2[n,o] per-partition bias free -> t2b; m2 = t2b*cb3
    (host-shipped selector tile, replaces v2's S3/B2 matmuls); R3
    reduce -> pyT [12,512] -> DMA out via gpsimd queue.

DMA: scalar queue issues the pipeline head (blobs, coT, xrep/corep q0)
starting ~3us into the NEFF (before sync's preamble ends); sync queue
carries the rest in strict consumption order (xr/cor quarters, then cb
pairs 0-3 block-interleaved, cb3). PE streams: 148 (~37.9us); DMA
~10.3MB (~35us at the ~300GB/s 16-engine cap); DVE ~35us; Act ~27us.
"""
import sys

sys.path.insert(0, "/opt/trn_rl_repo")

import numpy as np

import concourse.bacc as bacc
import concourse.bass as bass
import concourse.mybir as mybir
import concourse.tile as tile
from concourse.bass_utils import run_bass_kernel_spmd

N_CORES = 8
B = 65536
G = 10
CI, H, CO = 3, 64, 3
B_LOC = B // N_CORES

F32 = mybir.dt.float32
BF16 = mybir.dt.bfloat16


def host_constants(W0, W1, W2, b0, b1, b2):
    """Pack constants into bf16 blobs (cast on host; layout-only).

    blob42 [42, 320]: W0dup[0:128] | B1sel[128:192] | sel2[192:320]
    blob128 [128, 396]: W1s[0:320] | W2lo[320:352] | W2hi[352:384] | R3[384:396]
    b2vec [128, 1] f32: rows 32q+3n+o = b2[n,o] (Act eviction bias)
    """
    import ml_dtypes
    blob42 = np.zeros((42, 320), np.float32)
    W0dup = blob42[:, 0:128]
    B1sel = blob42[:, 128:192]
    sel2 = blob42[:, 192:320]      # rows 0:10 used: pair-4 cb broadcast
    sel2[8, 0:64] = 1.0
    sel2[9, 64:128] = 1.0
    for n in range(G):
        for i in range(CI):
            W0dup[n * 3 + i, 0:64] = W0[n, i]
            W0dup[n * 3 + i, 64:128] = W0[n, i]
        W0dup[32 + n, 0:64] = b0[n]
        W0dup[32 + n, 64:128] = b0[n]
        B1sel[32 + n] = b1[n]
    blob128 = np.zeros((128, 396), np.float32)
    W1s = blob128[:, 0:320]
    W2lo = blob128[0:64, 320:352]
    W2hi = blob128[64:128, 352:384]
    R3 = blob128[:, 384:396]
    for p in range(5):
        W1s[:H, p * H:(p + 1) * H] = W1[2 * p]
        W1s[H:, p * H:(p + 1) * H] = W1[2 * p + 1]
    for n in range(G):
        for o in range(CO):
            W2lo[:, n * 3 + o] = W2[n, :, o]
            W2hi[:, n * 3 + o] = W2[n, :, o]
    for c in range(4):
        for n in range(G):
            for o in range(CO):
                R3[32 * c + n * 3 + o, c * 3 + o] = 1.0
    b2vec = np.zeros((128, 1), np.float32)
    for q in range(4):
        for n in range(G):
            for o in range(CO):
                b2vec[32 * q + n * 3 + o, 0] = b2[n, o]
    return dict(
        blob42=blob42.astype(ml_dtypes.bfloat16),
        blob128=blob128.astype(ml_dtypes.bfloat16),
        b2vec=b2vec,
    )


def make_reps(x_loc, co_loc, b_loc=B_LOC):
    """Host-side zero-flop replication: feature-major row-replicated x and co
    in u0coT row layout (rows n*3+i -> x_i / co_n; rows 32+n -> 1 / co_n)."""
    import ml_dtypes
    S = b_loc // 128
    xT = x_loc.reshape(128, S, CI).transpose(2, 1, 0).reshape(CI, b_loc)
    coT = co_loc.reshape(128, S, G).transpose(2, 1, 0).reshape(G, b_loc)
    xrep = np.zeros((42, b_loc), np.float32)
    corep = np.zeros((42, b_loc), np.float32)
    for n in range(G):
        for i in range(CI):
            xrep[n * 3 + i] = xT[i]
            corep[n * 3 + i] = coT[n]
        xrep[32 + n] = 1.0
        corep[32 + n] = coT[n]
    return xrep.astype(ml_dtypes.bfloat16), corep.astype(ml_dtypes.bfloat16)


def make_cbs(co_loc, b_loc=B_LOC):
    """Host-side zero-flop layout prep (bf16 cast only):
    cb4 [4, 128, b_loc]: pair p rows 0:64 = co_{2p}, 64:128 = co_{2p+1}
      in u0coT column order (col = s*128 + p <-> sample b = p*S + s).
    cb3 [128, b_loc//4]: block g at cols g*512; rows 32q+3n+o = co_n of
      chunk 4g+q's columns (the L2 co-selector).
    coT [10, b_loc]: compact co for the on-device pair-4 broadcast.
    """
    import ml_dtypes
    S = b_loc // 128
    arr = co_loc.astype(ml_dtypes.bfloat16)          # [b_loc, 10]
    coT = arr.reshape(128, S, G).transpose(2, 1, 0).reshape(G, b_loc)
    cb4 = np.empty((4, 128, b_loc), dtype=ml_dtypes.bfloat16)
    for p in range(4):
        cb4[p, :64] = coT[2 * p]
        cb4[p, 64:] = coT[2 * p + 1]
    cb3 = np.zeros((128, b_loc // 4), dtype=ml_dtypes.bfloat16)
    for g in range(b_loc // 2048):
        for q in range(4):
            c = 4 * g + q
            for n in range(G):
                row = 32 * q + 3 * n
                seg = coT[n, c * 512:(c + 1) * 512]
                for o in range(CO):
                    cb3[row + o, g * 512:(g + 1) * 512] = seg
    return cb4, cb3, np.ascontiguousarray(coT)


def build(nc, b_loc=B_LOC):
    CHUNKS = b_loc // 512      # 512-col chunks (16)
    BLOCKS = CHUNKS // 4       # 2048-col blocks (4); L2 groups == blocks
    DT = 2048

    xr_d = nc.declare_dram_parameter("xrep", [42, b_loc], BF16, isOutput=False)
    cor_d = nc.declare_dram_parameter("corep", [42, b_loc], BF16, isOutput=False)
    b42_d = nc.declare_dram_parameter("blob42", [42, 320], BF16, isOutput=False)
    b128_d = nc.declare_dram_parameter("blob128", [128, 396], BF16, isOutput=False)
    b2v_d = nc.declare_dram_parameter("b2vec", [128, 1], F32, isOutput=False)
    cb_d = nc.declare_dram_parameter("cb4", [4, 128, b_loc], BF16, isOutput=False)
    cb3_d = nc.declare_dram_parameter("cb3", [128, b_loc // 4], BF16, isOutput=False)
    coT_d = nc.declare_dram_parameter("coT", [10, b_loc], BF16, isOutput=False)
    out_d = nc.declare_dram_parameter("out", [12, b_loc // 4], F32, isOutput=True)

    with tile.TileContext(nc) as tc:
        with (
            tc.tile_pool(name="consts", bufs=1) as consts,
            tc.tile_pool(name="chain", bufs=1) as chain,
            tc.tile_pool(name="xcq", bufs=3) as xcq_pool,
            tc.tile_pool(name="cotq", bufs=2) as cot_pool,
            tc.tile_pool(name="cbs", bufs=16) as cbs_pool,
            tc.tile_pool(name="xps", bufs=10) as xps_pool,
            tc.tile_pool(name="l2s", bufs=2) as l2s,
            tc.tile_pool(name="psZ", bufs=2, space="PSUM") as psZ,
            tc.tile_pool(name="psL1", bufs=2, space="PSUM") as psL1,
            tc.tile_pool(name="psT", bufs=2, space="PSUM") as psT,
            tc.tile_pool(name="psCB", bufs=2, space="PSUM") as psCB,
        ):
            # ---- small consts on the scalar queue (lands on early Q14) ----
            b42 = consts.tile([42, 320], BF16)
            nc.scalar.dma_start(b42[:], b42_d[:])
            b128 = consts.tile([128, 396], BF16)
            nc.scalar.dma_start(b128[:], b128_d[:])
            b2v = consts.tile([128, 1], F32)
            nc.scalar.dma_start(b2v[:], b2v_d[:])
            xr_q, cor_q, coT_q = [], [], []

            def load_xc(q, split=False):
                sl = slice(q * DT, (q + 1) * DT)
                xt = xcq_pool.tile([42, DT], BF16, tag="xr")
                ct = xcq_pool.tile([42, DT], BF16, tag="cor")
                if split:    # 1024-col halves so the first L0 starts sooner
                    nc.sync.dma_start(xt[:, 0:1024], xr_d[:, q * DT:q * DT + 1024])
                    nc.sync.dma_start(ct[:, 0:1024], cor_d[:, q * DT:q * DT + 1024])
                    nc.sync.dma_start(
                        xt[:, 1024:DT], xr_d[:, q * DT + 1024:(q + 1) * DT])
                    nc.sync.dma_start(
                        ct[:, 1024:DT], cor_d[:, q * DT + 1024:(q + 1) * DT])
                else:
                    nc.sync.dma_start(xt[:], xr_d[:, sl])
                    nc.sync.dma_start(ct[:], cor_d[:, sl])
                xr_q.append(xt)
                cor_q.append(ct)

            def load_coT(q):
                sl = slice(q * DT, (q + 1) * DT)
                t = cot_pool.tile([10, DT], BF16, tag="coT")
                nc.sync.dma_start(t[:], coT_d[:, sl])
                coT_q.append(t)

            load_xc(0, split=True)
            load_coT(0)

            W0dup = b42[:, 0:128]
            B1 = b42[:, 128:192]
            sel2 = b42[0:10, 192:320]
            W1s = b128[:, 0:320]
            W2lo = b128[:, 320:352]
            W2hi = b128[:, 352:384]
            R3 = b128[:, 384:396]

            # ---- sync queue: strict consumption order ----
            cb_tiles = {}       # (p, bblk) -> [128, DT] tile
            cb3 = chain.tile([128, b_loc // 4], BF16, tag="cb3")

            def load_cb(p, bblk):
                t = cbs_pool.tile([128, DT], BF16, tag="cb")
                nc.sync.dma_start(t[:], cb_d[p, :, bblk * DT:(bblk + 1) * DT])
                cb_tiles[(p, bblk)] = t

            # interleaved for arrival-order: cb(0,*) early for L1 block 0,
            # all xr/cor quarters early so the L0 chain never gates late
            # blocks, cb3 before R3(0)
            load_coT(1)
            load_xc(1)
            for p in range(2):
                load_cb(p, 0)
            load_xc(2)
            load_xc(3)
            load_cb(2, 0)
            load_cb(3, 0)
            nc.sync.dma_start(cb3[:], cb3_d[:])
            load_coT(2)
            load_coT(3)
            for p in range(4):
                load_cb(p, 1)
            for bblk in range(2, BLOCKS):
                for p in range(4):
                    load_cb(p, bblk)

            # ---- pair-4 cb via PE selector broadcast from compact coT ----
            cb4_sbs = []
            for bblk in range(BLOCKS):
                t = chain.tile([128, DT], BF16, tag=f"cb4_{bblk}")
                cb4_sbs.append(t)

            def emit_bc(c, evict):
                pcb4 = psCB.tile([128, 512], F32, tag="cb4ps")
                nc.tensor.matmul(
                    pcb4[:], sel2[:],
                    coT_q[c // 4][:, (c % 4) * 512:(c % 4) * 512 + 512],
                )
                dst = cb4_sbs[c // 4][:, (c % 4) * 512:(c % 4) * 512 + 512]
                if evict == "dve":
                    nc.vector.tensor_copy(dst, pcb4[:])
                elif evict == "gps":
                    nc.gpsimd.tensor_copy(dst, pcb4[:])
                else:
                    nc.scalar.activation(
                        dst, pcb4[:], mybir.ActivationFunctionType.Copy
                    )

            # ---- front: u0coT + L0 + pair-4 broadcast for blocks 0-1 ----
            u0coT = chain.tile([42, b_loc], BF16)
            x1dup = chain.tile([128, b_loc], BF16)
            x2dual = chain.tile([128, b_loc // 2], BF16)

            for c in range(CHUNKS):
                if c == 0 or c == 2:   # quarter 0 in 1024-col halves
                    h = slice((c // 2) * 1024, (c // 2) * 1024 + 1024)
                    nc.vector.tensor_tensor(
                        out=u0coT[:, h], in0=xr_q[0][:, h], in1=cor_q[0][:, h],
                        op=mybir.AluOpType.mult,
                    )
                elif c % 4 == 0:
                    q = c // 4
                    sl = slice(q * DT, (q + 1) * DT)
                    nc.vector.tensor_tensor(
                        out=u0coT[:, sl], in0=xr_q[q][:], in1=cor_q[q][:],
                        op=mybir.AluOpType.mult,
                    )
                pz = psZ.tile([128, 512], F32, tag="z")
                nc.tensor.matmul(pz[:], W0dup[:], u0coT[:, c * 512:(c + 1) * 512])
                nc.scalar.activation(
                    x1dup[:, c * 512:(c + 1) * 512], pz[:],
                    mybir.ActivationFunctionType.Relu,
                )
                if c < 4:
                    emit_bc(c, "dve")    # block-0 pair-4, ready before xp(0)
                elif c < 8:
                    emit_bc(c, "act")    # block-1 pair-4

            # ---- L2 split into stages so nothing head-of-line blocks:
            #      W2+t2b right after the block, m2 two blocks later (DVE
            #      idle slots), R3+out after the NEXT block's L1 ----
            pt2_t = {}
            t2b_t = {}
            m2_t = {}

            def emit_l2_mm(g):
                pt2 = psT.tile([128, 512], F32, tag="t2")
                for q in range(4):
                    c = 4 * g + q
                    d_abs, h = divmod(c, 2)
                    W2v = W2lo if h == 0 else W2hi
                    nc.tensor.matmul(
                        pt2[32 * q:32 * q + 32, :], W2v[:],
                        x2dual[:, d_abs * 512:(d_abs + 1) * 512],
                        tile_position=(0, 32 * q),
                        skip_group_check=True,
                    )
                t2b = l2s.tile([128, 512], BF16, tag="t2b")
                nc.scalar.activation(
                    t2b[:], pt2[:], mybir.ActivationFunctionType.Identity,
                    bias=b2v[:, 0:1],
                )
                t2b_t[g] = t2b

            def emit_m2(g):
                # blocks 0-2 on GpSimd (idle engine; keeps the DVE queue
                # free of head-of-line t2b waits); the last block on DVE
                # (DVE is free by then and GpSimd is slow on the tail)
                eng = nc.vector if g == BLOCKS - 1 else nc.gpsimd
                m2 = l2s.tile([128, 512], BF16, tag="m2")
                eng.tensor_tensor(
                    out=m2[:], in0=t2b_t[g][:],
                    in1=cb3[:, g * 512:(g + 1) * 512],
                    op=mybir.AluOpType.mult,
                )
                m2_t[g] = m2

            def emit_l2_fin(g):
                pyT = psZ.tile([12, 512], F32, tag="z")
                nc.tensor.matmul(pyT[:], R3[:], m2_t[g][:])
                yT_sb = l2s.tile([12, 512], F32, tag="yT")
                if g == BLOCKS - 1:      # tail chain: DVE is free, Act is not
                    nc.vector.tensor_copy(yT_sb[:], pyT[:])
                else:
                    nc.scalar.activation(
                        yT_sb[:], pyT[:], mybir.ActivationFunctionType.Copy
                    )
                nc.sync.dma_start(out_d[:, g * 512:(g + 1) * 512], yT_sb[:])

            # ---- L1 per 2048-block; pair order 4,0,1,2,3 so the last
            #      DMA arrival (pair 3) gates the least work ----
            PAIR_ORDER = (4, 0, 1, 2, 3)

            def emit_dd(bblk, dd, xps):
                pz2 = psL1.tile([128, 512], F32, tag="z2")
                for h in range(2):
                    c = 4 * bblk + 2 * dd + h
                    nc.tensor.matmul(
                        pz2[64 * h:64 * h + 64, :], B1[:],
                        u0coT[:, c * 512:(c + 1) * 512],
                        tile_position=(0, 64 * h),
                        start=True, stop=False,
                        skip_group_check=True,
                    )
                for pi, p in enumerate(PAIR_ORDER):
                    for h in range(2):
                        cc = 2 * dd + h
                        nc.tensor.matmul(
                            pz2[64 * h:64 * h + 64, :],
                            W1s[:, p * H:(p + 1) * H],
                            xps[p][:, cc * 512:(cc + 1) * 512],
                            tile_position=(0, 64 * h),
                            start=False, stop=(pi == 4),
                            skip_group_check=True,
                        )
                d_abs = 2 * bblk + dd
                nc.scalar.activation(
                    x2dual[:, d_abs * 512:(d_abs + 1) * 512], pz2[:],
                    mybir.ActivationFunctionType.Relu,
                )

            for bblk in range(BLOCKS):
                bsl = slice(bblk * DT, (bblk + 1) * DT)
                xps = {}
                for p in PAIR_ORDER:
                    xp = xps_pool.tile([128, DT], BF16, tag="xp")
                    in1 = cb4_sbs[bblk] if p == 4 else cb_tiles[(p, bblk)]
                    nc.vector.tensor_tensor(
                        out=xp[:], in0=x1dup[:, bsl], in1=in1[:],
                        op=mybir.AluOpType.mult,
                    )
                    xps[p] = xp
                if bblk >= 1:
                    emit_l2_mm(bblk - 1)
                if bblk >= 2:
                    emit_m2(bblk - 2)
                emit_dd(bblk, 0, xps)
                if bblk < 2:             # pair-4 bc for blocks 2-3, spread out
                    emit_bc(8 + 4 * bblk, "act")
                    emit_bc(9 + 4 * bblk, "act")
                emit_dd(bblk, 1, xps)
                if bblk < 2:
                    emit_bc(10 + 4 * bblk, "act")
                    emit_bc(11 + 4 * bblk, "act")
                if bblk >= 2:
                    emit_l2_fin(bblk - 2)
            emit_l2_mm(BLOCKS - 1)
            emit_m2(BLOCKS - 2)
            emit_l2_fin(BLOCKS - 2)
            emit_m2(BLOCKS - 1)
            emit_l2_fin(BLOCKS - 1)
    nc.compile()
    return nc


_NC_CACHE = {}


def get_nc(b_loc=B_LOC):
    if b_loc not in _NC_CACHE:
        nc = bacc.Bacc(None, target_bir_lowering=False)
        _NC_CACHE[b_loc] = build(nc, b_loc)
    return _NC_CACHE[b_loc]


def _unshuffle(yT, b_loc=B_LOC):
    """[12, b_loc/4] feature-major tiles -> [b_loc, 3] batch-major."""
    S = b_loc // 128
    y = np.empty((b_loc, CO), np.float32)
    r = np.arange(12)
    cq, o = r // 3, r % 3
    j = np.arange(512)
    for g in range(b_loc // DT_OUT):
        cg = (4 * g + cq[:, None]) * 512 + j[None, :]      # [12, 512] global col
        b_idx = (cg % 128) * S + cg // 128
        y[b_idx, np.broadcast_to(o[:, None], (12, 512))] = \
            yT[:, g * 512:(g + 1) * 512]
    return y


DT_OUT = 2048


def kernel(input, co_mat, W0, W1, W2, b0, b1, b2, _trace=False):
    input = np.asarray(input, np.float32)
    co_mat = np.asarray(co_mat, np.float32)
    consts = host_constants(
        np.asarray(W0, np.float32), np.asarray(W1, np.float32),
        np.asarray(W2, np.float32), np.asarray(b0, np.float32),
        np.asarray(b1, np.float32), np.asarray(b2, np.float32),
    )
    nc = get_nc()
    in_maps = []
    for k in range(N_CORES):
        sl = slice(k * B_LOC, (k + 1) * B_LOC)
        xr, cr = make_reps(input[sl], co_mat[sl])
        cb4, cb3, coT = make_cbs(co_mat[sl])
        m = {"xrep": xr, "corep": cr, "cb4": cb4, "cb3": cb3, "coT": coT}
        m.update(consts)
        in_maps.append(m)
    res = run_bass_kernel_spmd(
        nc, in_maps, core_ids=list(range(N_CORES)), trace=_trace
    )
    out = np.concatenate(
        [_unshuffle(res.results[k]["out"]) for k in range(N_CORES)], axis=0
    )
    if _trace:
        kernel.last_exec_time_ns = res.exec_time_ns
    return out


kernel.last_exec_time_ns = None
